# revision 24
# baseline (speedup 1.0000x reference)
"""Trainium2 Bass kernel for nn_DiffAttn (differential attention).

Reference computation (per batch b):
    Q = X @ Wq.T + bq ; K = X @ Wk.T + bk ; V = X @ Wv.T + bv
    Q1,Q2 / K1,K2 = halves of feature dim
    A_j = (Q_j @ K_j.T) / sqrt(DIM)
    out = softmax(A1) @ V - scalar * softmax(A2) @ V

Sharding: 8 cores = 4 batches x 2 token-halves. Each core projects
Q/K/V only for its OWN 1024 tokens; the K^T/V halves are exchanged
inside each batch pair with two pair-wise AllGather collectives
(HBM bounce buffers), so no projection work is duplicated. The gather
output is rank-ordered == key-half-ordered, so every core addresses
K/V tiles by global key index and the program is SPMD-uniform; the
only per-core data is the X^T token slab (and the host assembles the
output slabs).

All matmuls run in bf16 (fp32r moving operands measure ~2x slower per
column on TRN2 hardware): projections, scores, rowsums (ones-matmul),
and attn@V. P = exp(scores) is stored bf16; attention weights are
normalized BEFORE the V matmul (A = P1/r1 - scalar*P2/r2, with 1/r =
exp(-ln r) on the Scalar engine) so a single attn@V GEMM suffices.
The two query chunks are software-pipelined: the DVE combine of chunk
0 overlaps the score matmuls of chunk 1, and rowsum matmuls trail the
score chains by two tiles so the Scalar-engine exp latency stays off
the PE critical path. Output is written bf16 and widened on the host.
"""

import json
import math
from contextlib import ExitStack

import numpy as np
import ml_dtypes

import concourse.bass as bass
import concourse.tile as tile
from concourse import mybir
from concourse.bass_utils import run_bass_kernel_spmd


def _split_waits(raw: bytes, max_waits: int = 1) -> bytes:
    """walrus's CoreV3 codegen rejects instructions carrying more than one
    sync wait ("Too many sync wait commands"); Tile's kernel-tail drain
    aggregates one wait per live processor. Hoist excess waits onto chained
    same-engine Drain instructions inserted immediately before the offender."""
    m = json.loads(raw)
    uid = 0
    for fn in m["functions"]:
        for blk in fn["blocks"]:
            out = []
            for ins in blk["instructions"]:
                sy = ins.get("sync_info") or {}
                waits = sy.get("on_wait") or []
                if len(waits) > max_waits:
                    head, keep = waits[:-max_waits], waits[-max_waits:]
                    while head:
                        chunk, head = head[:max_waits], head[max_waits:]
                        uid += 1
                        out.append(
                            {
                                "engine": ins["engine"],
                                "ins": [],
                                "is_reset_sema": False,
                                "name": f"{ins['name']}-wsplit{uid}",
                                "opcode": "Drain",
                                "outs": [],
                                "sync_info": {"on_update": [], "on_wait": chunk},
                            }
                        )
                    sy["on_wait"] = keep
                out.append(ins)
            blk["instructions"] = out
    return json.dumps(m).encode()


B, S, DIM = 4, 2048, 1024
H = DIM // 2
NCORES = 8
TOK = S // 2           # tokens (queries and keys) owned per core
SCALE = 1.0 / math.sqrt(DIM)

BF16 = mybir.dt.bfloat16
F32 = mybir.dt.float32

DT = DIM // 128        # 8  contraction tiles over model dim
CT = DIM // 128        # 8  feature tiles of Q^T/K^T
KT = S // 128          # 16 key tiles (global)
KTH = TOK // 128       # 8  key tiles per core half
NQC = TOK // 512       # 2  query chunks of 512

# test harness hooks (the grader never touches these)
TRACE = False
LAST_RESULTS = None


def _build_bass():
    nc = bass.Bass(
        trn_type="TRN2",
        target_bir_lowering=False,
        debug=False,
        num_devices=NCORES,
    )

    xth = nc.dram_tensor("xth", [DIM, TOK], BF16, kind="ExternalInput")
    wqt = nc.dram_tensor("wqt", [DIM, DIM], BF16, kind="ExternalInput")
    wkt = nc.dram_tensor("wkt", [DIM, DIM], BF16, kind="ExternalInput")
    wvt = nc.dram_tensor("wvt", [DIM, DIM], BF16, kind="ExternalInput")
    bqr = nc.dram_tensor("bqr", [128, CT], F32, kind="ExternalInput")
    bkr = nc.dram_tensor("bkr", [128, CT], F32, kind="ExternalInput")
    bvb = nc.dram_tensor("bvb", [128, DIM], F32, kind="ExternalInput")
    scv = nc.dram_tensor("scv", [128, 1], F32, kind="ExternalInput")
    outp = nc.dram_tensor("out", [TOK, DIM], BF16, kind="ExternalOutput")

    Id = mybir.ActivationFunctionType.Identity
    Exp = mybir.ActivationFunctionType.Exp
    Ln = mybir.ActivationFunctionType.Ln
    PAIRS = [[0, 1], [2, 3], [4, 5], [6, 7]]

    with tile.TileContext(nc) as tc, ExitStack() as ctx:
        const = ctx.enter_context(tc.tile_pool(name="const", bufs=1))
        persist = ctx.enter_context(tc.tile_pool(name="persist", bufs=1))
        dram = ctx.enter_context(tc.tile_pool(name="dram", bufs=1, space="DRAM"))
        ps_s = ctx.enter_context(tc.tile_pool(name="ps_s", bufs=6, space="PSUM"))

        bq_sb = const.tile([128, CT], F32)
        nc.sync.dma_start(out=bq_sb[:, :], in_=bqr[:, :])
        bk_sb = const.tile([128, CT], F32)
        nc.sync.dma_start(out=bk_sb[:, :], in_=bkr[:, :])
        sc_sb = const.tile([128, 1], F32)
        nc.sync.dma_start(out=sc_sb[:, :], in_=scv[:, :])
        ones_sb = const.tile([128, 2], F32)
        nc.vector.memset(ones_sb[:, :], 1.0)

        # Warm the PE clock gate (HAM) during the initial input-DMA wait:
        # a chain of tiny dependent matmuls gives ~4.5 us of sustained PE
        # activity so the first projection matmuls run at 2.4 GHz, not 1.2.
        with tc.psum_pool(name="ps_w", bufs=1) as ps_w:
            warm = ps_w.tile([2, 2], F32, name="warm")
            for _ in range(64):
                nc.tensor.matmul(
                    warm[:, :], ones_sb[:, :], ones_sb[:, :], start=True, stop=True
                )

        # ones stationary for the rowsum matmul (column sums, replicated
        # across partitions)
        ones_f = const.tile([128, 128], F32)
        nc.vector.memset(ones_f[:, :], 1.0)
        ones_bf = const.tile([128, 128], BF16)
        nc.vector.tensor_copy(ones_bf[:, :], ones_f[:, :])
        lnsc_sb = const.tile([128, 1], F32)
        nc.scalar.activation(lnsc_sb[:, :], sc_sb[:, :], Ln)

        # persistent products
        q_sb = [persist.tile([128, TOK], BF16, name=f"q{i}") for i in range(CT)]
        k_sb = [persist.tile([128, S], BF16, name=f"k{i}") for i in range(CT)]
        v_sb = [persist.tile([128, DIM], BF16, name=f"v{i}") for i in range(KT)]

        # DRAM bounce buffers for the pair exchange. The K gather is split
        # in two so the first feature half starts flying while the second
        # is still projecting (and before the CC-stream prelude barrier
        # would otherwise serialize one big transfer).
        k_send = [dram.tile([128, 4 * TOK], BF16, name=f"k_send{h}") for h in range(2)]
        k_recv = [
            dram.tile([2, 128, 4 * TOK], BF16, name=f"k_recv{h}") for h in range(2)
        ]
        v_send = dram.tile([128, KTH * DIM], BF16, name="v_send")
        v_recv = dram.tile([2, 128, KTH * DIM], BF16, name="v_recv")

        # XT tiles feed K, V and Q projections
        xtp = tc.alloc_tile_pool(name="xtp", bufs=1)
        x_t = [xtp.tile([128, TOK], BF16, name=f"x{d}") for d in range(DT)]

        # All input loads issue upfront on the sync-engine queue; staging
        # sends ride the scalar-engine HWDGE queue so they flow at compute
        # pace instead of FIFO-ing behind the bulk input transfers, and the
        # sync engine's blocking waits on the collective outputs never gate
        # any other issue.
        weights = tc.alloc_tile_pool(name="weights", bufs=1)
        wk_t = [weights.tile([128, DIM], BF16, name=f"wk{d}") for d in range(DT)]
        wv_t = [weights.tile([128, DIM], BF16, name=f"wv{d}") for d in range(DT)]
        wq_t = [weights.tile([128, DIM], BF16, name=f"wq{d}") for d in range(DT)]
        bv_sb = weights.tile([128, DIM], F32, name="bv_sb")
        # First K-proj tiles ride the scalar-engine queue: both DMA rings
        # init in parallel (~7 us each) and the first chain starts sooner.
        for d in range(2):
            nc.scalar.dma_start(out=x_t[d][:, :], in_=xth[d * 128 : (d + 1) * 128, :])
            nc.scalar.dma_start(out=wk_t[d][:, :], in_=wkt[d * 128 : (d + 1) * 128, :])
        for d in range(2, DT):
            nc.sync.dma_start(out=x_t[d][:, :], in_=xth[d * 128 : (d + 1) * 128, :])
            nc.sync.dma_start(out=wk_t[d][:, :], in_=wkt[d * 128 : (d + 1) * 128, :])
        for d in range(DT):
            nc.sync.dma_start(out=wv_t[d][:, :], in_=wvt[d * 128 : (d + 1) * 128, :])
        nc.sync.dma_start(out=bv_sb[:, :], in_=bvb[:, :])
        for d in range(DT):
            nc.sync.dma_start(out=wq_t[d][:, :], in_=wqt[d * 128 : (d + 1) * 128, :])

        # ---- Phase K: K^T[c, own keys] = Wk^T.T @ X^T  (+bk) ----
        kstage = tc.alloc_tile_pool(name="kstage", bufs=1)
        k_st = [kstage.tile([128, TOK], BF16, name=f"ks{c}") for c in range(CT)]
        for c in range(CT):
            for n in range(TOK // 512):
                ps = ps_s.tile([128, 512], F32, tag="ps", name="psk")
                for d in range(DT):
                    nc.tensor.matmul(
                        ps[:, :],
                        wk_t[d][:, c * 128 : (c + 1) * 128],
                        x_t[d][:, n * 512 : (n + 1) * 512],
                        start=(d == 0),
                        stop=(d == DT - 1),
                    )
                nc.scalar.activation(
                    k_st[c][:, n * 512 : (n + 1) * 512],
                    ps[:, :],
                    Id,
                    bias=bk_sb[:, c : c + 1],
                )
            nc.scalar.dma_start(
                out=k_send[c // 4][:, (c % 4) * TOK : (c % 4 + 1) * TOK],
                in_=k_st[c][:, :],
            )
            if c % 4 == 3:
                # pair-wise AllGather: K^T feature half (rank order ==
                # key-half order)
                nc.gpsimd.collective_compute(
                    "AllGather",
                    mybir.AluOpType.bypass,
                    replica_groups=PAIRS,
                    ins=[k_send[c // 4][:, :].opt()],
                    outs=[k_recv[c // 4][:, :, :].opt()],
                )

        for h in range(2):
            for g in range(2):
                for ci in range(4):
                    c = 4 * h + ci
                    nc.sync.dma_start(
                        out=k_sb[c][:, g * TOK : (g + 1) * TOK],
                        in_=k_recv[h][g, :, ci * TOK : (ci + 1) * TOK],
                    )

        # ---- Phase V: V[own keys, :] = X^T.T @ Wv^T  (+bv) ----
        vstage = tc.alloc_tile_pool(name="vstage", bufs=1)
        v_st = [vstage.tile([128, DIM], BF16, name=f"vs{i}") for i in range(KTH)]
        for i in range(KTH):
            for n in range(DIM // 512):
                ps = ps_s.tile([128, 512], F32, tag="ps", name="psv")
                for d in range(DT):
                    nc.tensor.matmul(
                        ps[:, :],
                        x_t[d][:, i * 128 : (i + 1) * 128],
                        wv_t[d][:, n * 512 : (n + 1) * 512],
                        start=(d == 0),
                        stop=(d == DT - 1),
                    )
                nc.vector.tensor_add(
                    v_st[i][:, n * 512 : (n + 1) * 512],
                    ps[:, :],
                    bv_sb[:, n * 512 : (n + 1) * 512],
                )
            nc.scalar.dma_start(
                out=v_send[:, i * DIM : (i + 1) * DIM], in_=v_st[i][:, :]
            )

        # pair-wise AllGather: V halves
        nc.gpsimd.collective_compute(
            "AllGather",
            mybir.AluOpType.bypass,
            replica_groups=PAIRS,
            ins=[v_send[:, :].opt()],
            outs=[v_recv[:, :, :].opt()],
        )
        for g in range(2):
            for i in range(KTH):
                nc.sync.dma_start(
                    out=v_sb[g * KTH + i][:, :],
                    in_=v_recv[g, :, i * DIM : (i + 1) * DIM],
                )

        # ---- Phase Q: Q^T[c, own queries] = Wq^T.T @ X^T  (+bq) ----
        for c in range(CT):
            for n in range(TOK // 512):
                ps = ps_s.tile([128, 512], F32, tag="ps", name="psq")
                for d in range(DT):
                    nc.tensor.matmul(
                        ps[:, :],
                        wq_t[d][:, c * 128 : (c + 1) * 128],
                        x_t[d][:, n * 512 : (n + 1) * 512],
                        start=(d == 0),
                        stop=(d == DT - 1),
                    )
                nc.scalar.activation(
                    q_sb[c][:, n * 512 : (n + 1) * 512],
                    ps[:, :],
                    Id,
                    bias=bq_sb[:, c : c + 1],
                )
        vstage.release()
        kstage.release()
        weights.release()
        xtp.release()

        # ---- Phase 2: attention ----
        # P^T tiles for both query chunks stay live so chunk 1's score
        # matmuls overlap chunk 0's DVE combine.
        with (
            tc.tile_pool(name="pP", bufs=1) as pP,
            tc.tile_pool(name="ps_r", bufs=2, space="PSUM") as ps_r,
            tc.tile_pool(name="small", bufs=1) as small,
            tc.tile_pool(name="tmp2", bufs=2) as tmp2,
            tc.tile_pool(name="ostage", bufs=3) as ostage,
        ):
            p_sb = [
                [
                    [pP.tile([128, 512], BF16, name=f"p{qc}_{j}_{k}") for k in range(KT)]
                    for j in range(2)
                ]
                for qc in range(NQC)
            ]
            bcs = [[None, None] for _ in range(NQC)]
            rps = {}

            def emit_group(qc, j, deferred):
                # scores S^T[k, q] = K_j^T.T @ Q_j^T; P = exp(s*S^T);
                # r = column sums via bf16 ones-matmul, lagging the score
                # chains by 2 tiles so the Scalar exp latency stays off the
                # PE critical path. The last two rowsums + Ln/bc of the
                # PREVIOUS group are deferred into this group (after chain
                # k=1) for the same reason.
                r_ps = ps_r.tile([128, 512], F32, tag="r", name=f"r{qc}{j}")
                rps[(qc, j)] = r_ps
                for k in range(KT):
                    ps = ps_s.tile([128, 512], F32, tag="ps", name="pss")
                    for ci in range(4):
                        c = 4 * j + ci
                        nc.tensor.matmul(
                            ps[:, :],
                            k_sb[c][:, k * 128 : (k + 1) * 128],
                            q_sb[c][:, qc * 512 : (qc + 1) * 512],
                            start=(ci == 0),
                            stop=(ci == 3),
                        )
                    nc.scalar.activation(
                        p_sb[qc][j][k][:, :], ps[:, :], Exp, scale=SCALE
                    )
                    if k == 1 and deferred is not None:
                        emit_rowsum(*deferred)
                    if k >= 2:
                        nc.tensor.matmul(
                            r_ps[:, :],
                            ones_bf[:, :],
                            p_sb[qc][j][k - 2][:, :],
                            start=(k == 2),
                            stop=False,
                        )

            def emit_rowsum(qc, j):
                # tail rowsums of group (qc, j), then bc_j = exp(-ln r_j) =
                # 1/r_j on the Scalar engine (j=1 folds the input scalar in
                # via a +ln(scalar) bias).
                r_ps = rps[(qc, j)]
                for k in (KT - 2, KT - 1):
                    nc.tensor.matmul(
                        r_ps[:, :],
                        ones_bf[:, :],
                        p_sb[qc][j][k][:, :],
                        start=False,
                        stop=(k == KT - 1),
                    )
                lnr = tmp2.tile([128, 512], F32, tag="lnr", name="lnr")
                nc.scalar.activation(lnr[:, :], r_ps[:, :], Ln)
                bc = small.tile([128, 512], BF16, tag=f"bc{qc}{j}", name=f"bc{qc}{j}")
                if j == 0:
                    nc.scalar.activation(bc[:, :], lnr[:, :], Exp, scale=-1.0)
                else:
                    nc.scalar.activation(
                        bc[:, :], lnr[:, :], Exp, scale=-1.0, bias=lnsc_sb[:, :]
                    )
                bcs[qc][j] = bc

            def emit_combine(qc):
                # A^T[k] = P1[k]*bc1 - P2[k]*bc2s  (in place into p_sb[qc][1])
                for k in range(KT):
                    t2 = tmp2.tile([128, 512], BF16, tag="t2", name="t2")
                    nc.vector.tensor_mul(t2[:, :], p_sb[qc][0][k][:, :], bcs[qc][0][:, :])
                    nc.vector.tensor_mul(
                        p_sb[qc][1][k][:, :], p_sb[qc][1][k][:, :], bcs[qc][1][:, :]
                    )
                    nc.vector.tensor_sub(
                        p_sb[qc][1][k][:, :], t2[:, :], p_sb[qc][1][k][:, :]
                    )

            def emit_attnv(qc, chains=None):
                # out rows = A^T.T @ V
                for t, n in chains or [(t, n) for t in range(4) for n in range(2)]:
                    row = qc * 512 + t * 128
                    if True:
                        lo, hi = n * 512, (n + 1) * 512
                        u = ps_s.tile([128, 512], F32, tag="ps", name="u")
                        for k in range(KT):
                            nc.tensor.matmul(
                                u[:, :],
                                p_sb[qc][1][k][:, t * 128 : (t + 1) * 128],
                                v_sb[k][:, lo:hi],
                                start=(k == 0),
                                stop=(k == KT - 1),
                            )
                        o = ostage.tile([128, 512], BF16, tag="o", name="o")
                        nc.scalar.copy(o[:, :], u[:, :])
                        nc.sync.dma_start(out=outp[row : row + 128, lo:hi], in_=o[:, :])

            # j-major group order: the two j=0 groups only need the first
            # K-gather (features 0..511), giving the second gather until the
            # third group to land.
            emit_group(0, 0, None)
            emit_group(1, 0, (0, 0))
            emit_group(0, 1, (1, 0))
            emit_group(1, 1, (0, 1))
            emit_combine(0)
            emit_attnv(0, chains=[(0, 0)])
            emit_rowsum(1, 1)
            emit_combine(1)
            emit_attnv(0, chains=[(t, n) for t in range(4) for n in range(2)][1:])
            emit_attnv(1)

    return nc


_NC_CACHE = None


def _get_nc():
    global _NC_CACHE
    if _NC_CACHE is None:
        nc = _build_bass()
        fixed = _split_waits(bass.Bass.to_json_bytes(nc))
        nc.to_json_bytes = lambda: fixed
        _NC_CACHE = nc
    return _NC_CACHE


def kernel(hidden_states, W_q, b_q, W_k, b_k, W_v, b_v, scalar):
    global LAST_RESULTS
    bf16 = ml_dtypes.bfloat16
    X = np.asarray(hidden_states, np.float32)
    wqt = np.ascontiguousarray(np.asarray(W_q, np.float32).T).astype(bf16)
    wkt = np.ascontiguousarray(np.asarray(W_k, np.float32).T).astype(bf16)
    wvt = np.ascontiguousarray(np.asarray(W_v, np.float32).T).astype(bf16)
    bqr = np.ascontiguousarray(np.asarray(b_q, np.float32).reshape(CT, 128).T)
    bkr = np.ascontiguousarray(np.asarray(b_k, np.float32).reshape(CT, 128).T)
    bvb = np.ascontiguousarray(
        np.broadcast_to(np.asarray(b_v, np.float32), (128, DIM))
    )
    scv = np.full((128, 1), np.asarray(scalar, np.float32).reshape(-1)[0], np.float32)

    in_maps = []
    for core in range(NCORES):
        b, h = core // 2, core % 2
        xt_b = np.ascontiguousarray(X[b, h * TOK : (h + 1) * TOK, :].T).astype(bf16)
        in_maps.append(
            {
                "xth": xt_b,
                "wqt": wqt,
                "wkt": wkt,
                "wvt": wvt,
                "bqr": bqr,
                "bkr": bkr,
                "bvb": bvb,
                "scv": scv,
            }
        )

    nc = _get_nc()
    res = run_bass_kernel_spmd(
        nc,
        in_maps,
        list(range(NCORES)),
        trace=TRACE,
    )
    LAST_RESULTS = res

    out = np.empty((B, S, DIM), np.float32)
    for core in range(NCORES):
        b, h = core // 2, core % 2
        out[b, h * TOK : (h + 1) * TOK, :] = res.results[core]["out"].astype(np.float32)
    return out


if __name__ == "__main__":
    import reference

    inputs = {k: np.asarray(v) for k, v in reference.setup_inputs().items()}
    got = kernel(**inputs)
    print("kernel output", got.shape, got.dtype)


# revision 31
# speedup vs baseline: 1.0053x; 1.0053x over previous
"""Trainium2 Bass kernel for nn_DiffAttn (differential attention).

Reference computation (per batch b):
    Q = X @ Wq.T + bq ; K = X @ Wk.T + bk ; V = X @ Wv.T + bv
    Q1,Q2 / K1,K2 = halves of feature dim
    A_j = (Q_j @ K_j.T) / sqrt(DIM)
    out = softmax(A1) @ V - scalar * softmax(A2) @ V

Sharding: 8 cores = 4 batches x 2 token-halves. Each core projects
Q/K/V only for its OWN 1024 tokens; the K^T/V halves are exchanged
inside each batch pair with two pair-wise AllGather collectives
(HBM bounce buffers), so no projection work is duplicated. The gather
output is rank-ordered == key-half-ordered, so every core addresses
K/V tiles by global key index and the program is SPMD-uniform; the
only per-core data is the X^T token slab (and the host assembles the
output slabs).

All matmuls run in bf16 (fp32r moving operands measure ~2x slower per
column on TRN2 hardware): projections, scores, rowsums (ones-matmul),
and attn@V. P = exp(scores) is stored bf16; attention weights are
normalized BEFORE the V matmul (A = P1/r1 - scalar*P2/r2, with 1/r =
exp(-ln r) on the Scalar engine) so a single attn@V GEMM suffices.
The two query chunks are software-pipelined: the DVE combine of chunk
0 overlaps the score matmuls of chunk 1, and rowsum matmuls trail the
score chains by two tiles so the Scalar-engine exp latency stays off
the PE critical path. Output is written bf16 and widened on the host.
"""

import json
import math
from contextlib import ExitStack

import numpy as np
import ml_dtypes

import concourse.bass as bass
import concourse.tile as tile
from concourse import mybir
from concourse.bass_utils import run_bass_kernel_spmd


def _split_waits(raw: bytes, max_waits: int = 1) -> bytes:
    """walrus's CoreV3 codegen rejects instructions carrying more than one
    sync wait ("Too many sync wait commands"); Tile's kernel-tail drain
    aggregates one wait per live processor. Hoist excess waits onto chained
    same-engine Drain instructions inserted immediately before the offender."""
    m = json.loads(raw)
    uid = 0
    for fn in m["functions"]:
        for blk in fn["blocks"]:
            out = []
            for ins in blk["instructions"]:
                sy = ins.get("sync_info") or {}
                waits = sy.get("on_wait") or []
                if len(waits) > max_waits:
                    head, keep = waits[:-max_waits], waits[-max_waits:]
                    while head:
                        chunk, head = head[:max_waits], head[max_waits:]
                        uid += 1
                        out.append(
                            {
                                "engine": ins["engine"],
                                "ins": [],
                                "is_reset_sema": False,
                                "name": f"{ins['name']}-wsplit{uid}",
                                "opcode": "Drain",
                                "outs": [],
                                "sync_info": {"on_update": [], "on_wait": chunk},
                            }
                        )
                    sy["on_wait"] = keep
                out.append(ins)
            blk["instructions"] = out
    return json.dumps(m).encode()


B, S, DIM = 4, 2048, 1024
H = DIM // 2
NCORES = 8
TOK = S // 2           # tokens (queries and keys) owned per core
SCALE = 1.0 / math.sqrt(DIM)

BF16 = mybir.dt.bfloat16
F32 = mybir.dt.float32

DT = DIM // 128        # 8  contraction tiles over model dim
CT = DIM // 128        # 8  feature tiles of Q^T/K^T
KT = S // 128          # 16 key tiles (global)
KTH = TOK // 128       # 8  key tiles per core half
NQC = TOK // 512       # 2  query chunks of 512

# test harness hooks (the grader never touches these)
TRACE = False
LAST_RESULTS = None


def _build_bass():
    nc = bass.Bass(
        trn_type="TRN2",
        target_bir_lowering=False,
        debug=False,
        num_devices=NCORES,
    )

    xth = nc.dram_tensor("xth", [DIM, TOK], BF16, kind="ExternalInput")
    wqt = nc.dram_tensor("wqt", [DIM, DIM], BF16, kind="ExternalInput")
    wkt = nc.dram_tensor("wkt", [DIM, DIM], BF16, kind="ExternalInput")
    wvt = nc.dram_tensor("wvt", [DIM, DIM], BF16, kind="ExternalInput")
    bqr = nc.dram_tensor("bqr", [128, CT], F32, kind="ExternalInput")
    bkr = nc.dram_tensor("bkr", [128, CT], F32, kind="ExternalInput")
    bvb = nc.dram_tensor("bvb", [128, DIM], F32, kind="ExternalInput")
    scv = nc.dram_tensor("scv", [128, 1], F32, kind="ExternalInput")
    outp = nc.dram_tensor("out", [TOK, DIM], BF16, kind="ExternalOutput")

    Id = mybir.ActivationFunctionType.Identity
    Exp = mybir.ActivationFunctionType.Exp
    Ln = mybir.ActivationFunctionType.Ln
    PAIRS = [[0, 1], [2, 3], [4, 5], [6, 7]]

    with tile.TileContext(nc) as tc, ExitStack() as ctx:
        const = ctx.enter_context(tc.tile_pool(name="const", bufs=1))
        persist = ctx.enter_context(tc.tile_pool(name="persist", bufs=1))
        dram = ctx.enter_context(tc.tile_pool(name="dram", bufs=1, space="DRAM"))
        ps_s = ctx.enter_context(tc.tile_pool(name="ps_s", bufs=6, space="PSUM"))

        bq_sb = const.tile([128, CT], F32)
        nc.sync.dma_start(out=bq_sb[:, :], in_=bqr[:, :])
        bk_sb = const.tile([128, CT], F32)
        nc.sync.dma_start(out=bk_sb[:, :], in_=bkr[:, :])
        sc_sb = const.tile([128, 1], F32)
        nc.sync.dma_start(out=sc_sb[:, :], in_=scv[:, :])
        ones_sb = const.tile([128, 2], F32)
        nc.vector.memset(ones_sb[:, :], 1.0)

        # Warm the PE clock gate (HAM) during the initial input-DMA wait:
        # a chain of tiny dependent matmuls gives ~4.5 us of sustained PE
        # activity so the first projection matmuls run at 2.4 GHz, not 1.2.
        with tc.psum_pool(name="ps_w", bufs=1) as ps_w:
            warm = ps_w.tile([2, 2], F32, name="warm")
            for _ in range(96):
                nc.tensor.matmul(
                    warm[:, :], ones_sb[:, :], ones_sb[:, :], start=True, stop=True
                )

        # ones stationary for the rowsum matmul (column sums, replicated
        # across partitions)
        ones_f = const.tile([128, 128], F32)
        nc.vector.memset(ones_f[:, :], 1.0)
        ones_bf = const.tile([128, 128], BF16)
        nc.vector.tensor_copy(ones_bf[:, :], ones_f[:, :])
        lnsc_sb = const.tile([128, 1], F32)
        nc.scalar.activation(lnsc_sb[:, :], sc_sb[:, :], Ln)

        # persistent products
        q_sb = [persist.tile([128, TOK], BF16, name=f"q{i}") for i in range(CT)]
        k_sb = [persist.tile([128, S], BF16, name=f"k{i}") for i in range(CT)]
        v_sb = [persist.tile([128, DIM], BF16, name=f"v{i}") for i in range(KT)]

        # DRAM bounce buffers for the pair exchange. The K gather is split
        # in two so the first feature half starts flying while the second
        # is still projecting (and before the CC-stream prelude barrier
        # would otherwise serialize one big transfer).
        k_send = [dram.tile([128, 4 * TOK], BF16, name=f"k_send{h}") for h in range(2)]
        k_recv = [
            dram.tile([2, 128, 4 * TOK], BF16, name=f"k_recv{h}") for h in range(2)
        ]
        v_send = dram.tile([128, KTH * DIM], BF16, name="v_send")
        v_recv = dram.tile([2, 128, KTH * DIM], BF16, name="v_recv")

        # XT tiles feed K, V and Q projections
        xtp = tc.alloc_tile_pool(name="xtp", bufs=1)
        x_t = [xtp.tile([128, TOK], BF16, name=f"x{d}") for d in range(DT)]

        # All input loads issue upfront on the sync-engine queue; staging
        # sends ride the scalar-engine HWDGE queue so they flow at compute
        # pace instead of FIFO-ing behind the bulk input transfers, and the
        # sync engine's blocking waits on the collective outputs never gate
        # any other issue.
        weights = tc.alloc_tile_pool(name="weights", bufs=1)
        wk_t = [weights.tile([128, DIM], BF16, name=f"wk{d}") for d in range(DT)]
        wv_t = [weights.tile([128, DIM], BF16, name=f"wv{d}") for d in range(DT)]
        wq_t = [weights.tile([128, DIM], BF16, name=f"wq{d}") for d in range(DT)]
        bv_sb = weights.tile([128, DIM], F32, name="bv_sb")
        for d in range(DT):
            nc.sync.dma_start(out=x_t[d][:, :], in_=xth[d * 128 : (d + 1) * 128, :])
            nc.sync.dma_start(out=wk_t[d][:, :], in_=wkt[d * 128 : (d + 1) * 128, :])
        for d in range(DT):
            nc.sync.dma_start(out=wv_t[d][:, :], in_=wvt[d * 128 : (d + 1) * 128, :])
        nc.sync.dma_start(out=bv_sb[:, :], in_=bvb[:, :])
        for d in range(DT):
            nc.sync.dma_start(out=wq_t[d][:, :], in_=wqt[d * 128 : (d + 1) * 128, :])

        # ---- Phase K: K^T[c, own keys] = Wk^T.T @ X^T  (+bk) ----
        kstage = tc.alloc_tile_pool(name="kstage", bufs=1)
        k_st = [kstage.tile([128, TOK], BF16, name=f"ks{c}") for c in range(CT)]
        for c in range(CT):
            for n in range(TOK // 512):
                ps = ps_s.tile([128, 512], F32, tag="ps", name="psk")
                for d in range(DT):
                    nc.tensor.matmul(
                        ps[:, :],
                        wk_t[d][:, c * 128 : (c + 1) * 128],
                        x_t[d][:, n * 512 : (n + 1) * 512],
                        start=(d == 0),
                        stop=(d == DT - 1),
                    )
                nc.scalar.activation(
                    k_st[c][:, n * 512 : (n + 1) * 512],
                    ps[:, :],
                    Id,
                    bias=bk_sb[:, c : c + 1],
                )
            nc.scalar.dma_start(
                out=k_send[c // 4][:, (c % 4) * TOK : (c % 4 + 1) * TOK],
                in_=k_st[c][:, :],
            )
            if c % 4 == 3:
                # pair-wise AllGather: K^T feature half (rank order ==
                # key-half order)
                nc.gpsimd.collective_compute(
                    "AllGather",
                    mybir.AluOpType.bypass,
                    replica_groups=PAIRS,
                    ins=[k_send[c // 4][:, :].opt()],
                    outs=[k_recv[c // 4][:, :, :].opt()],
                )

        for h in range(2):
            for g in range(2):
                for ci in range(4):
                    c = 4 * h + ci
                    nc.sync.dma_start(
                        out=k_sb[c][:, g * TOK : (g + 1) * TOK],
                        in_=k_recv[h][g, :, ci * TOK : (ci + 1) * TOK],
                    )

        # ---- Phase V: V[own keys, :] = X^T.T @ Wv^T  (+bv) ----
        vstage = tc.alloc_tile_pool(name="vstage", bufs=1)
        v_st = [vstage.tile([128, DIM], BF16, name=f"vs{i}") for i in range(KTH)]
        for i in range(KTH):
            for n in range(DIM // 512):
                ps = ps_s.tile([128, 512], F32, tag="ps", name="psv")
                for d in range(DT):
                    nc.tensor.matmul(
                        ps[:, :],
                        x_t[d][:, i * 128 : (i + 1) * 128],
                        wv_t[d][:, n * 512 : (n + 1) * 512],
                        start=(d == 0),
                        stop=(d == DT - 1),
                    )
                nc.vector.tensor_add(
                    v_st[i][:, n * 512 : (n + 1) * 512],
                    ps[:, :],
                    bv_sb[:, n * 512 : (n + 1) * 512],
                )
            nc.scalar.dma_start(
                out=v_send[:, i * DIM : (i + 1) * DIM], in_=v_st[i][:, :]
            )

        # pair-wise AllGather: V halves
        nc.gpsimd.collective_compute(
            "AllGather",
            mybir.AluOpType.bypass,
            replica_groups=PAIRS,
            ins=[v_send[:, :].opt()],
            outs=[v_recv[:, :, :].opt()],
        )
        for g in range(2):
            for i in range(KTH):
                nc.sync.dma_start(
                    out=v_sb[g * KTH + i][:, :],
                    in_=v_recv[g, :, i * DIM : (i + 1) * DIM],
                )

        # ---- Phase Q: Q^T[c, own queries] = Wq^T.T @ X^T  (+bq) ----
        for c in range(CT):
            for n in range(TOK // 512):
                ps = ps_s.tile([128, 512], F32, tag="ps", name="psq")
                for d in range(DT):
                    nc.tensor.matmul(
                        ps[:, :],
                        wq_t[d][:, c * 128 : (c + 1) * 128],
                        x_t[d][:, n * 512 : (n + 1) * 512],
                        start=(d == 0),
                        stop=(d == DT - 1),
                    )
                nc.scalar.activation(
                    q_sb[c][:, n * 512 : (n + 1) * 512],
                    ps[:, :],
                    Id,
                    bias=bq_sb[:, c : c + 1],
                )
        vstage.release()
        kstage.release()
        weights.release()
        xtp.release()

        # ---- Phase 2: attention ----
        # P^T tiles for both query chunks stay live so chunk 1's score
        # matmuls overlap chunk 0's DVE combine.
        with (
            tc.tile_pool(name="pP", bufs=1) as pP,
            tc.tile_pool(name="ps_r", bufs=2, space="PSUM") as ps_r,
            tc.tile_pool(name="small", bufs=1) as small,
            tc.tile_pool(name="tmp2", bufs=2) as tmp2,
            tc.tile_pool(name="ostage", bufs=3) as ostage,
        ):
            p_sb = [
                [
                    [pP.tile([128, 512], BF16, name=f"p{qc}_{j}_{k}") for k in range(KT)]
                    for j in range(2)
                ]
                for qc in range(NQC)
            ]
            bcs = [[None, None] for _ in range(NQC)]
            rps = {}

            def emit_group(qc, j, deferred):
                # scores S^T[k, q] = K_j^T.T @ Q_j^T; P = exp(s*S^T);
                # r = column sums via bf16 ones-matmul, lagging the score
                # chains by 2 tiles so the Scalar exp latency stays off the
                # PE critical path. The last two rowsums + Ln/bc of the
                # PREVIOUS group are deferred into this group (after chain
                # k=1) for the same reason.
                r_ps = ps_r.tile([128, 512], F32, tag="r", name=f"r{qc}{j}")
                rps[(qc, j)] = r_ps
                for k in range(KT):
                    ps = ps_s.tile([128, 512], F32, tag="ps", name="pss")
                    for ci in range(4):
                        c = 4 * j + ci
                        nc.tensor.matmul(
                            ps[:, :],
                            k_sb[c][:, k * 128 : (k + 1) * 128],
                            q_sb[c][:, qc * 512 : (qc + 1) * 512],
                            start=(ci == 0),
                            stop=(ci == 3),
                        )
                    nc.scalar.activation(
                        p_sb[qc][j][k][:, :], ps[:, :], Exp, scale=SCALE
                    )
                    if k == 1 and deferred is not None:
                        emit_rowsum(*deferred)
                    if k >= 2:
                        nc.tensor.matmul(
                            r_ps[:, :],
                            ones_bf[:, :],
                            p_sb[qc][j][k - 2][:, :],
                            start=(k == 2),
                            stop=False,
                        )

            def emit_rowsum(qc, j):
                # tail rowsums of group (qc, j), then bc_j = exp(-ln r_j) =
                # 1/r_j on the Scalar engine (j=1 folds the input scalar in
                # via a +ln(scalar) bias).
                r_ps = rps[(qc, j)]
                for k in (KT - 2, KT - 1):
                    nc.tensor.matmul(
                        r_ps[:, :],
                        ones_bf[:, :],
                        p_sb[qc][j][k][:, :],
                        start=False,
                        stop=(k == KT - 1),
                    )
                lnr = tmp2.tile([128, 512], F32, tag="lnr", name="lnr")
                nc.scalar.activation(lnr[:, :], r_ps[:, :], Ln)
                bc = small.tile([128, 512], BF16, tag=f"bc{qc}{j}", name=f"bc{qc}{j}")
                if j == 0:
                    nc.scalar.activation(bc[:, :], lnr[:, :], Exp, scale=-1.0)
                else:
                    nc.scalar.activation(
                        bc[:, :], lnr[:, :], Exp, scale=-1.0, bias=lnsc_sb[:, :]
                    )
                bcs[qc][j] = bc

            def emit_combine(qc):
                # A^T[k] = P1[k]*bc1 - P2[k]*bc2s  (in place into p_sb[qc][1])
                for k in range(KT):
                    t2 = tmp2.tile([128, 512], BF16, tag="t2", name="t2")
                    nc.vector.tensor_mul(t2[:, :], p_sb[qc][0][k][:, :], bcs[qc][0][:, :])
                    nc.vector.tensor_mul(
                        p_sb[qc][1][k][:, :], p_sb[qc][1][k][:, :], bcs[qc][1][:, :]
                    )
                    nc.vector.tensor_sub(
                        p_sb[qc][1][k][:, :], t2[:, :], p_sb[qc][1][k][:, :]
                    )

            def emit_attnv(qc, chains=None):
                # out rows = A^T.T @ V
                for t, n in chains or [(t, n) for t in range(4) for n in range(2)]:
                    row = qc * 512 + t * 128
                    if True:
                        lo, hi = n * 512, (n + 1) * 512
                        u = ps_s.tile([128, 512], F32, tag="ps", name="u")
                        for k in range(KT):
                            nc.tensor.matmul(
                                u[:, :],
                                p_sb[qc][1][k][:, t * 128 : (t + 1) * 128],
                                v_sb[k][:, lo:hi],
                                start=(k == 0),
                                stop=(k == KT - 1),
                            )
                        o = ostage.tile([128, 512], BF16, tag="o", name="o")
                        nc.scalar.copy(o[:, :], u[:, :])
                        nc.sync.dma_start(out=outp[row : row + 128, lo:hi], in_=o[:, :])

            # j-major group order: the two j=0 groups only need the first
            # K-gather (features 0..511), giving the second gather until the
            # third group to land.
            emit_group(0, 0, None)
            emit_group(1, 0, (0, 0))
            emit_group(0, 1, (1, 0))
            emit_group(1, 1, (0, 1))
            emit_combine(0)
            emit_attnv(0, chains=[(0, 0)])
            emit_rowsum(1, 1)
            emit_combine(1)
            emit_attnv(0, chains=[(t, n) for t in range(4) for n in range(2)][1:])
            emit_attnv(1)

    return nc


_NC_CACHE = None


def _get_nc():
    global _NC_CACHE
    if _NC_CACHE is None:
        nc = _build_bass()
        fixed = _split_waits(bass.Bass.to_json_bytes(nc))
        nc.to_json_bytes = lambda: fixed
        _NC_CACHE = nc
    return _NC_CACHE


def kernel(hidden_states, W_q, b_q, W_k, b_k, W_v, b_v, scalar):
    global LAST_RESULTS
    bf16 = ml_dtypes.bfloat16
    X = np.asarray(hidden_states, np.float32)
    wqt = np.ascontiguousarray(np.asarray(W_q, np.float32).T).astype(bf16)
    wkt = np.ascontiguousarray(np.asarray(W_k, np.float32).T).astype(bf16)
    wvt = np.ascontiguousarray(np.asarray(W_v, np.float32).T).astype(bf16)
    bqr = np.ascontiguousarray(np.asarray(b_q, np.float32).reshape(CT, 128).T)
    bkr = np.ascontiguousarray(np.asarray(b_k, np.float32).reshape(CT, 128).T)
    bvb = np.ascontiguousarray(
        np.broadcast_to(np.asarray(b_v, np.float32), (128, DIM))
    )
    scv = np.full((128, 1), np.asarray(scalar, np.float32).reshape(-1)[0], np.float32)

    in_maps = []
    for core in range(NCORES):
        b, h = core // 2, core % 2
        xt_b = np.ascontiguousarray(X[b, h * TOK : (h + 1) * TOK, :].T).astype(bf16)
        in_maps.append(
            {
                "xth": xt_b,
                "wqt": wqt,
                "wkt": wkt,
                "wvt": wvt,
                "bqr": bqr,
                "bkr": bkr,
                "bvb": bvb,
                "scv": scv,
            }
        )

    nc = _get_nc()
    res = run_bass_kernel_spmd(
        nc,
        in_maps,
        list(range(NCORES)),
        trace=TRACE,
    )
    LAST_RESULTS = res

    out = np.empty((B, S, DIM), np.float32)
    for core in range(NCORES):
        b, h = core // 2, core % 2
        out[b, h * TOK : (h + 1) * TOK, :] = res.results[core]["out"].astype(np.float32)
    return out


if __name__ == "__main__":
    import reference

    inputs = {k: np.asarray(v) for k, v in reference.setup_inputs().items()}
    got = kernel(**inputs)
    print("kernel output", got.shape, got.dtype)


# revision 32
# speedup vs baseline: 1.0093x; 1.0040x over previous
"""Trainium2 Bass kernel for nn_DiffAttn (differential attention).

Reference computation (per batch b):
    Q = X @ Wq.T + bq ; K = X @ Wk.T + bk ; V = X @ Wv.T + bv
    Q1,Q2 / K1,K2 = halves of feature dim
    A_j = (Q_j @ K_j.T) / sqrt(DIM)
    out = softmax(A1) @ V - scalar * softmax(A2) @ V

Sharding: 8 cores = 4 batches x 2 token-halves. Each core projects
Q/K/V only for its OWN 1024 tokens; the K^T/V halves are exchanged
inside each batch pair with two pair-wise AllGather collectives
(HBM bounce buffers), so no projection work is duplicated. The gather
output is rank-ordered == key-half-ordered, so every core addresses
K/V tiles by global key index and the program is SPMD-uniform; the
only per-core data is the X^T token slab (and the host assembles the
output slabs).

All matmuls run in bf16 (fp32r moving operands measure ~2x slower per
column on TRN2 hardware): projections, scores, rowsums (ones-matmul),
and attn@V. P = exp(scores) is stored bf16; attention weights are
normalized BEFORE the V matmul (A = P1/r1 - scalar*P2/r2, with 1/r =
exp(-ln r) on the Scalar engine) so a single attn@V GEMM suffices.
The two query chunks are software-pipelined: the DVE combine of chunk
0 overlaps the score matmuls of chunk 1, and rowsum matmuls trail the
score chains by two tiles so the Scalar-engine exp latency stays off
the PE critical path. Output is written bf16 and widened on the host.
"""

import json
import math
from contextlib import ExitStack

import numpy as np
import ml_dtypes

import concourse.bass as bass
import concourse.tile as tile
from concourse import mybir
from concourse.bass_utils import run_bass_kernel_spmd


def _split_waits(raw: bytes, max_waits: int = 1) -> bytes:
    """walrus's CoreV3 codegen rejects instructions carrying more than one
    sync wait ("Too many sync wait commands"); Tile's kernel-tail drain
    aggregates one wait per live processor. Hoist excess waits onto chained
    same-engine Drain instructions inserted immediately before the offender."""
    m = json.loads(raw)
    uid = 0
    for fn in m["functions"]:
        for blk in fn["blocks"]:
            out = []
            for ins in blk["instructions"]:
                sy = ins.get("sync_info") or {}
                waits = sy.get("on_wait") or []
                if len(waits) > max_waits:
                    head, keep = waits[:-max_waits], waits[-max_waits:]
                    while head:
                        chunk, head = head[:max_waits], head[max_waits:]
                        uid += 1
                        out.append(
                            {
                                "engine": ins["engine"],
                                "ins": [],
                                "is_reset_sema": False,
                                "name": f"{ins['name']}-wsplit{uid}",
                                "opcode": "Drain",
                                "outs": [],
                                "sync_info": {"on_update": [], "on_wait": chunk},
                            }
                        )
                    sy["on_wait"] = keep
                out.append(ins)
            blk["instructions"] = out
    return json.dumps(m).encode()


B, S, DIM = 4, 2048, 1024
H = DIM // 2
NCORES = 8
TOK = S // 2           # tokens (queries and keys) owned per core
SCALE = 1.0 / math.sqrt(DIM)

BF16 = mybir.dt.bfloat16
F32 = mybir.dt.float32

DT = DIM // 128        # 8  contraction tiles over model dim
CT = DIM // 128        # 8  feature tiles of Q^T/K^T
KT = S // 128          # 16 key tiles (global)
KTH = TOK // 128       # 8  key tiles per core half
NQC = TOK // 512       # 2  query chunks of 512

# test harness hooks (the grader never touches these)
TRACE = False
LAST_RESULTS = None


def _build_bass():
    nc = bass.Bass(
        trn_type="TRN2",
        target_bir_lowering=False,
        debug=False,
        num_devices=NCORES,
    )

    xth = nc.dram_tensor("xth", [DIM, TOK], BF16, kind="ExternalInput")
    wqt = nc.dram_tensor("wqt", [DIM, DIM], BF16, kind="ExternalInput")
    wkt = nc.dram_tensor("wkt", [DIM, DIM], BF16, kind="ExternalInput")
    wvt = nc.dram_tensor("wvt", [DIM, DIM], BF16, kind="ExternalInput")
    bqr = nc.dram_tensor("bqr", [128, CT], F32, kind="ExternalInput")
    bkr = nc.dram_tensor("bkr", [128, CT], F32, kind="ExternalInput")
    bvb = nc.dram_tensor("bvb", [128, DIM], F32, kind="ExternalInput")
    scv = nc.dram_tensor("scv", [128, 1], F32, kind="ExternalInput")
    outp = nc.dram_tensor("out", [TOK, DIM], BF16, kind="ExternalOutput")

    Id = mybir.ActivationFunctionType.Identity
    Exp = mybir.ActivationFunctionType.Exp
    Ln = mybir.ActivationFunctionType.Ln
    PAIRS = [[0, 1], [2, 3], [4, 5], [6, 7]]

    with tile.TileContext(nc) as tc, ExitStack() as ctx:
        const = ctx.enter_context(tc.tile_pool(name="const", bufs=1))
        persist = ctx.enter_context(tc.tile_pool(name="persist", bufs=1))
        dram = ctx.enter_context(tc.tile_pool(name="dram", bufs=1, space="DRAM"))
        ps_s = ctx.enter_context(tc.tile_pool(name="ps_s", bufs=6, space="PSUM"))

        bq_sb = const.tile([128, CT], F32)
        nc.sync.dma_start(out=bq_sb[:, :], in_=bqr[:, :])
        bk_sb = const.tile([128, CT], F32)
        nc.sync.dma_start(out=bk_sb[:, :], in_=bkr[:, :])
        sc_sb = const.tile([128, 1], F32)
        nc.sync.dma_start(out=sc_sb[:, :], in_=scv[:, :])
        ones_sb = const.tile([128, 2], F32)
        nc.vector.memset(ones_sb[:, :], 1.0)

        # Warm the PE clock gate (HAM) during the initial input-DMA wait:
        # a chain of tiny dependent matmuls gives ~4.5 us of sustained PE
        # activity so the first projection matmuls run at 2.4 GHz, not 1.2.
        with tc.psum_pool(name="ps_w", bufs=1) as ps_w:
            warm = ps_w.tile([2, 2], F32, name="warm")
            for _ in range(64):
                nc.tensor.matmul(
                    warm[:, :], ones_sb[:, :], ones_sb[:, :], start=True, stop=True
                )

        # ones stationary for the rowsum matmul (column sums, replicated
        # across partitions)
        ones_f = const.tile([128, 128], F32)
        nc.vector.memset(ones_f[:, :], 1.0)
        ones_bf = const.tile([128, 128], BF16)
        nc.vector.tensor_copy(ones_bf[:, :], ones_f[:, :])
        lnsc_sb = const.tile([128, 1], F32)
        nc.scalar.activation(lnsc_sb[:, :], sc_sb[:, :], Ln)

        # persistent products
        q_sb = [persist.tile([128, TOK], BF16, name=f"q{i}") for i in range(CT)]
        k_sb = [persist.tile([128, S], BF16, name=f"k{i}") for i in range(CT)]
        v_sb = [persist.tile([128, DIM], BF16, name=f"v{i}") for i in range(KT)]

        # DRAM bounce buffers for the pair exchange. The K gather is split
        # in two so the first feature half starts flying while the second
        # is still projecting (and before the CC-stream prelude barrier
        # would otherwise serialize one big transfer).
        k_send = [dram.tile([128, 4 * TOK], BF16, name=f"k_send{h}") for h in range(2)]
        k_recv = [
            dram.tile([2, 128, 4 * TOK], BF16, name=f"k_recv{h}") for h in range(2)
        ]
        v_send = dram.tile([128, KTH * DIM], BF16, name="v_send")
        v_recv = dram.tile([2, 128, KTH * DIM], BF16, name="v_recv")

        # XT tiles feed K, V and Q projections
        xtp = tc.alloc_tile_pool(name="xtp", bufs=1)
        x_t = [xtp.tile([128, TOK], BF16, name=f"x{d}") for d in range(DT)]

        # All input loads issue upfront on the sync-engine queue; staging
        # sends ride the scalar-engine HWDGE queue so they flow at compute
        # pace instead of FIFO-ing behind the bulk input transfers, and the
        # sync engine's blocking waits on the collective outputs never gate
        # any other issue.
        weights = tc.alloc_tile_pool(name="weights", bufs=1)
        wk_t = [weights.tile([128, DIM], BF16, name=f"wk{d}") for d in range(DT)]
        wv_t = [weights.tile([128, DIM], BF16, name=f"wv{d}") for d in range(DT)]
        wq_t = [weights.tile([128, DIM], BF16, name=f"wq{d}") for d in range(DT)]
        bv_sb = weights.tile([128, DIM], F32, name="bv_sb")
        for d in range(DT):
            nc.sync.dma_start(out=x_t[d][:, :], in_=xth[d * 128 : (d + 1) * 128, :])
            nc.sync.dma_start(out=wk_t[d][:, :], in_=wkt[d * 128 : (d + 1) * 128, :])
        for d in range(DT):
            nc.sync.dma_start(out=wv_t[d][:, :], in_=wvt[d * 128 : (d + 1) * 128, :])
        nc.sync.dma_start(out=bv_sb[:, :], in_=bvb[:, :])
        for d in range(DT):
            nc.sync.dma_start(out=wq_t[d][:, :], in_=wqt[d * 128 : (d + 1) * 128, :])

        # ---- Phase K: K^T[c, own keys] = Wk^T.T @ X^T  (+bk) ----
        kstage = tc.alloc_tile_pool(name="kstage", bufs=1)
        k_st = [kstage.tile([128, TOK], BF16, name=f"ks{c}") for c in range(CT)]
        for c in range(CT):
            for n in range(TOK // 512):
                ps = ps_s.tile([128, 512], F32, tag="ps", name="psk")
                for d in range(DT):
                    nc.tensor.matmul(
                        ps[:, :],
                        wk_t[d][:, c * 128 : (c + 1) * 128],
                        x_t[d][:, n * 512 : (n + 1) * 512],
                        start=(d == 0),
                        stop=(d == DT - 1),
                    )
                nc.scalar.activation(
                    k_st[c][:, n * 512 : (n + 1) * 512],
                    ps[:, :],
                    Id,
                    bias=bk_sb[:, c : c + 1],
                )
            nc.scalar.dma_start(
                out=k_send[c // 4][:, (c % 4) * TOK : (c % 4 + 1) * TOK],
                in_=k_st[c][:, :],
            )
            if c % 4 == 3:
                # pair-wise AllGather: K^T feature half (rank order ==
                # key-half order)
                nc.gpsimd.collective_compute(
                    "AllGather",
                    mybir.AluOpType.bypass,
                    replica_groups=PAIRS,
                    ins=[k_send[c // 4][:, :].opt()],
                    outs=[k_recv[c // 4][:, :, :].opt()],
                )

        for h in range(2):
            for g in range(2):
                for ci in range(4):
                    c = 4 * h + ci
                    nc.sync.dma_start(
                        out=k_sb[c][:, g * TOK : (g + 1) * TOK],
                        in_=k_recv[h][g, :, ci * TOK : (ci + 1) * TOK],
                    )

        # ---- Phase V: V[own keys, :] = X^T.T @ Wv^T  (+bv) ----
        vstage = tc.alloc_tile_pool(name="vstage", bufs=1)
        v_st = [vstage.tile([128, DIM], BF16, name=f"vs{i}") for i in range(KTH)]
        for i in range(KTH):
            for n in range(DIM // 512):
                ps = ps_s.tile([128, 512], F32, tag="ps", name="psv")
                for d in range(DT):
                    nc.tensor.matmul(
                        ps[:, :],
                        x_t[d][:, i * 128 : (i + 1) * 128],
                        wv_t[d][:, n * 512 : (n + 1) * 512],
                        start=(d == 0),
                        stop=(d == DT - 1),
                    )
                nc.vector.tensor_add(
                    v_st[i][:, n * 512 : (n + 1) * 512],
                    ps[:, :],
                    bv_sb[:, n * 512 : (n + 1) * 512],
                )
            nc.scalar.dma_start(
                out=v_send[:, i * DIM : (i + 1) * DIM], in_=v_st[i][:, :]
            )

        # pair-wise AllGather: V halves
        nc.gpsimd.collective_compute(
            "AllGather",
            mybir.AluOpType.bypass,
            replica_groups=PAIRS,
            ins=[v_send[:, :].opt()],
            outs=[v_recv[:, :, :].opt()],
        )
        for g in range(2):
            for i in range(KTH):
                nc.sync.dma_start(
                    out=v_sb[g * KTH + i][:, :],
                    in_=v_recv[g, :, i * DIM : (i + 1) * DIM],
                )

        # ---- Phase Q: Q^T[c, own queries] = Wq^T.T @ X^T  (+bq) ----
        for c in range(CT):
            for n in range(TOK // 512):
                ps = ps_s.tile([128, 512], F32, tag="ps", name="psq")
                for d in range(DT):
                    nc.tensor.matmul(
                        ps[:, :],
                        wq_t[d][:, c * 128 : (c + 1) * 128],
                        x_t[d][:, n * 512 : (n + 1) * 512],
                        start=(d == 0),
                        stop=(d == DT - 1),
                    )
                nc.scalar.activation(
                    q_sb[c][:, n * 512 : (n + 1) * 512],
                    ps[:, :],
                    Id,
                    bias=bq_sb[:, c : c + 1],
                )
        vstage.release()
        kstage.release()
        weights.release()
        xtp.release()

        # ---- Phase 2: attention ----
        # P^T tiles for both query chunks stay live so chunk 1's score
        # matmuls overlap chunk 0's DVE combine.
        with (
            tc.tile_pool(name="pP", bufs=1) as pP,
            tc.tile_pool(name="ps_r", bufs=2, space="PSUM") as ps_r,
            tc.tile_pool(name="small", bufs=1) as small,
            tc.tile_pool(name="tmp2", bufs=2) as tmp2,
            tc.tile_pool(name="ostage", bufs=3) as ostage,
        ):
            p_sb = [
                [
                    [pP.tile([128, 512], BF16, name=f"p{qc}_{j}_{k}") for k in range(KT)]
                    for j in range(2)
                ]
                for qc in range(NQC)
            ]
            bcs = [[None, None] for _ in range(NQC)]
            rps = {}

            def emit_group(qc, j, deferred):
                # scores S^T[k, q] = K_j^T.T @ Q_j^T; P = exp(s*S^T);
                # r = column sums via bf16 ones-matmul, lagging the score
                # chains by 2 tiles so the Scalar exp latency stays off the
                # PE critical path. The last two rowsums + Ln/bc of the
                # PREVIOUS group are deferred into this group (after chain
                # k=1) for the same reason.
                r_ps = ps_r.tile([128, 512], F32, tag="r", name=f"r{qc}{j}")
                rps[(qc, j)] = r_ps
                for k in range(KT):
                    ps = ps_s.tile([128, 512], F32, tag="ps", name="pss")
                    for ci in range(4):
                        c = 4 * j + ci
                        nc.tensor.matmul(
                            ps[:, :],
                            k_sb[c][:, k * 128 : (k + 1) * 128],
                            q_sb[c][:, qc * 512 : (qc + 1) * 512],
                            start=(ci == 0),
                            stop=(ci == 3),
                        )
                    nc.scalar.activation(
                        p_sb[qc][j][k][:, :], ps[:, :], Exp, scale=SCALE
                    )
                    if k == 1 and deferred is not None:
                        emit_rowsum(*deferred)
                    if k >= 2:
                        nc.tensor.matmul(
                            r_ps[:, :],
                            ones_bf[:, :],
                            p_sb[qc][j][k - 2][:, :],
                            start=(k == 2),
                            stop=False,
                        )

            def emit_rowsum(qc, j):
                # tail rowsums of group (qc, j), then bc_j = exp(-ln r_j) =
                # 1/r_j on the Scalar engine (j=1 folds the input scalar in
                # via a +ln(scalar) bias).
                r_ps = rps[(qc, j)]
                for k in (KT - 2, KT - 1):
                    nc.tensor.matmul(
                        r_ps[:, :],
                        ones_bf[:, :],
                        p_sb[qc][j][k][:, :],
                        start=False,
                        stop=(k == KT - 1),
                    )
                lnr = tmp2.tile([128, 512], F32, tag="lnr", name="lnr")
                nc.scalar.activation(lnr[:, :], r_ps[:, :], Ln)
                bc = small.tile([128, 512], BF16, tag=f"bc{qc}{j}", name=f"bc{qc}{j}")
                if j == 0:
                    nc.scalar.activation(bc[:, :], lnr[:, :], Exp, scale=-1.0)
                else:
                    nc.scalar.activation(
                        bc[:, :], lnr[:, :], Exp, scale=-1.0, bias=lnsc_sb[:, :]
                    )
                bcs[qc][j] = bc

            def emit_combine(qc):
                # A^T[k] = P1[k]*bc1 - P2[k]*bc2s  (in place into p_sb[qc][1])
                for k in range(KT):
                    t2 = tmp2.tile([128, 512], BF16, tag="t2", name="t2")
                    nc.vector.tensor_mul(t2[:, :], p_sb[qc][0][k][:, :], bcs[qc][0][:, :])
                    nc.vector.tensor_mul(
                        p_sb[qc][1][k][:, :], p_sb[qc][1][k][:, :], bcs[qc][1][:, :]
                    )
                    nc.vector.tensor_sub(
                        p_sb[qc][1][k][:, :], t2[:, :], p_sb[qc][1][k][:, :]
                    )

            def emit_attnv(qc, chains=None):
                # out rows = A^T.T @ V
                for t, n in chains or [(t, n) for t in range(4) for n in range(2)]:
                    row = qc * 512 + t * 128
                    if True:
                        lo, hi = n * 512, (n + 1) * 512
                        u = ps_s.tile([128, 512], F32, tag="ps", name="u")
                        for k in range(KT):
                            nc.tensor.matmul(
                                u[:, :],
                                p_sb[qc][1][k][:, t * 128 : (t + 1) * 128],
                                v_sb[k][:, lo:hi],
                                start=(k == 0),
                                stop=(k == KT - 1),
                            )
                        o = ostage.tile([128, 512], BF16, tag="o", name="o")
                        nc.scalar.copy(o[:, :], u[:, :])
                        nc.sync.dma_start(out=outp[row : row + 128, lo:hi], in_=o[:, :])

            # j-major group order: the two j=0 groups only need the first
            # K-gather (features 0..511), giving the second gather until the
            # third group to land.
            emit_group(0, 0, None)
            emit_group(1, 0, (0, 0))
            emit_group(0, 1, (1, 0))
            emit_group(1, 1, (0, 1))
            emit_combine(0)
            emit_attnv(0, chains=[(0, 0)])
            emit_rowsum(1, 1)
            emit_combine(1)
            emit_attnv(0, chains=[(t, n) for t in range(4) for n in range(2)][1:])
            emit_attnv(1)

    return nc


_NC_CACHE = None


def _get_nc():
    global _NC_CACHE
    if _NC_CACHE is None:
        nc = _build_bass()
        fixed = _split_waits(bass.Bass.to_json_bytes(nc))
        nc.to_json_bytes = lambda: fixed
        _NC_CACHE = nc
    return _NC_CACHE


def kernel(hidden_states, W_q, b_q, W_k, b_k, W_v, b_v, scalar):
    global LAST_RESULTS
    bf16 = ml_dtypes.bfloat16
    X = np.asarray(hidden_states, np.float32)
    wqt = np.ascontiguousarray(np.asarray(W_q, np.float32).T).astype(bf16)
    wkt = np.ascontiguousarray(np.asarray(W_k, np.float32).T).astype(bf16)
    wvt = np.ascontiguousarray(np.asarray(W_v, np.float32).T).astype(bf16)
    bqr = np.ascontiguousarray(np.asarray(b_q, np.float32).reshape(CT, 128).T)
    bkr = np.ascontiguousarray(np.asarray(b_k, np.float32).reshape(CT, 128).T)
    bvb = np.ascontiguousarray(
        np.broadcast_to(np.asarray(b_v, np.float32), (128, DIM))
    )
    scv = np.full((128, 1), np.asarray(scalar, np.float32).reshape(-1)[0], np.float32)

    in_maps = []
    for core in range(NCORES):
        b, h = core // 2, core % 2
        xt_b = np.ascontiguousarray(X[b, h * TOK : (h + 1) * TOK, :].T).astype(bf16)
        in_maps.append(
            {
                "xth": xt_b,
                "wqt": wqt,
                "wkt": wkt,
                "wvt": wvt,
                "bqr": bqr,
                "bkr": bkr,
                "bvb": bvb,
                "scv": scv,
            }
        )

    nc = _get_nc()
    res = run_bass_kernel_spmd(
        nc,
        in_maps,
        list(range(NCORES)),
        trace=TRACE,
    )
    LAST_RESULTS = res

    out = np.empty((B, S, DIM), np.float32)
    for core in range(NCORES):
        b, h = core // 2, core % 2
        out[b, h * TOK : (h + 1) * TOK, :] = res.results[core]["out"].astype(np.float32)
    return out


if __name__ == "__main__":
    import reference

    inputs = {k: np.asarray(v) for k, v in reference.setup_inputs().items()}
    got = kernel(**inputs)
    print("kernel output", got.shape, got.dtype)
